# revision 42
# baseline (speedup 1.0000x reference)
"""Block-local attention (B=20, S=4096, E=256, BLOCK=128, 1 block halo each side)
for 8 trn2 NeuronCores.

Strategy:
  - Fold weights on host: Mq = Wq @ Wk.T / sqrt(E), Wvo = Wv @ Wo.
    logits = X Mq X^T, out = softmax(logits) @ (X Wvo). Saves the separate
    K projection and the output projection entirely.
  - Shard: 40 shards = (batch, half-sequence of 16 blocks); 5 shards/core.
    Each shard ships pre-transposed X^T (fp16) with 1 halo block each side
    (zeros at batch edges -> zero-pad softmax semantics match the reference
    exactly: padded keys contribute exp(0) to the denominator, V=0).
  - Per query block j: logits computed TRANSPOSED (kpos x q) so exp(logitsT)
    is directly the lhsT of the A@V' matmul -- no on-chip transposes.
    Softmax denominator = ones column appended to V' (free in the same
    matmuls). Normalization fused into the PSUM->SBUF copy of z.
  - All matmul operands fp16 (1 cycle/row on PE), fp32 PSUM accumulation,
    softmax + output in fp32.
"""

import numpy as np

try:
    import ml_dtypes
except ImportError:        # float16 comes from numpy anyway
    ml_dtypes = None

B, S, E = 20, 4096, 256
BLOCK = 128
NB = S // BLOCK          # 32 blocks per batch
HALF = 16                # query blocks per shard
CTX = HALF + 2           # blocks incl. halo
NCORES = 8
SHPC = 5                 # shards per core (40 shards total)
SHARD_Q = HALF * BLOCK   # 2048
SHARD_X = CTX * BLOCK    # 2304

F16 = np.dtype(np.float16)

_CACHE = {}


def _build_module(split=True, pq_bufs=1, pv_bufs=2, pl_bufs=1, pz_bufs=3, z_act_mod=2, xt_bufs=12, e_bufs=2, q_bufs=2, v_bufs=2, z_bufs=8, zgrp=2, qcopy_mod=0, prefetch_all=False, pool_mode='stack', warmup_mm=8, r_bufs=4):
    import concourse.bass as bass
    import concourse.tile as tile
    import concourse.mybir as mybir

    f16 = mybir.dt.float16
    f32 = mybir.dt.float32
    Exp = mybir.ActivationFunctionType.Exp
    Copy = mybir.ActivationFunctionType.Copy

    nc = bass.Bass("TRN2", target_bir_lowering=False, debug=False,
                   num_devices=NCORES)
    xt_d = nc.declare_dram_parameter("xt", [SHPC, 2, 128, SHARD_X], f16, isOutput=False)
    mq_d = nc.declare_dram_parameter("mq", [128, 4, 128], f16, isOutput=False)
    wvo_d = nc.declare_dram_parameter("wvo", [128, 2, 256], f16, isOutput=False)
    z_d = nc.declare_dram_parameter("z", [SHPC, HALF, 128, 256], f16, isOutput=True)

    mult = mybir.AluOpType.mult

    with tile.TileContext(nc, pool_alloc_mode=pool_mode) as tc:
        with (
            tc.tile_pool(name="wpool", bufs=1) as wpool,
            tc.tile_pool(name="xpool", bufs=xt_bufs) as xpool,
            tc.tile_pool(name="qpool", bufs=q_bufs) as qpool,
            tc.tile_pool(name="vpool", bufs=v_bufs) as vpool,
            tc.tile_pool(name="epool", bufs=e_bufs) as epool,
            tc.tile_pool(name="zpool", bufs=z_bufs) as zpool,
            tc.tile_pool(name="rpool", bufs=r_bufs) as rpool,
            tc.tile_pool(name="pq", bufs=pq_bufs, space="PSUM") as pqpool,
            tc.tile_pool(name="pv", bufs=pv_bufs, space="PSUM") as pvpool,
            tc.tile_pool(name="pl", bufs=pl_bufs, space="PSUM") as plpool,
            tc.tile_pool(name="pz", bufs=pz_bufs, space="PSUM") as pzpool,
        ):
            # weights: one strided DMA each, first in the SP / ACT queues
            # (small transfers; the PE warm-up and V' both gate on them)
            mq_sb = wpool.tile([128, 4, 128], f16)    # [:, ei*2+eo, :]
            nc.sync.dma_start(out=mq_sb[:], in_=mq_d[:])
            wvo_sb = wpool.tile([128, 2, 256], f16)

            # piece cuts are kb- and Q'-chunk aligned; shard 0 uses finer
            # pieces so the first transfers (which gate the pipeline fill)
            # land as early as possible
            XCUTS_FINE = [0, 640, 1152, 1664, SHARD_X]
            XCUTS_STD = [0, 1152, SHARD_X]

            if warmup_mm:
                # dummy matmuls to lift the PE out of its cold p-state under
                # the shadow of the first xt DMAs
                pwu = pqpool.tile([128, 512], f32, tag="pq")
                for i in range(warmup_mm):
                    nc.tensor.matmul(pwu[:, 0:128], lhsT=mq_sb[:, 0, :],
                                     rhs=mq_sb[:, 1, :],
                                     start=(i == 0), stop=(i == warmup_mm - 1))

            def load_shard(s):
                # shard 0: finer pieces, ei0 on the SP queue and ei1 on the
                # ACT queue so both queues generate descriptors in parallel
                # during the pipeline fill. Later shards: two halves on SP.
                cuts = XCUTS_FINE if s == 0 else XCUTS_STD
                xh = []
                for ei in range(2):
                    eng = (nc.sync if ei == 0 else nc.scalar) if s == 0 else nc.sync
                    ps = []
                    for p in range(len(cuts) - 1):
                        c0, c1 = cuts[p], cuts[p + 1]
                        t = xpool.tile([128, c1 - c0], f16, tag="xt")
                        eng.dma_start(out=t[:], in_=xt_d[s, ei, :, c0:c1])
                        ps.append(t)
                    xh.append(ps)
                return (cuts, xh)

            if prefetch_all:
                all_x = [load_shard(s) for s in range(SHPC)]

            for s in range(SHPC):
                xhalf = all_x[s] if prefetch_all else load_shard(s)
                if s == 0:
                    # deferred: behind shard-0 x pieces on the ACT queue so
                    # the ei1 pieces that gate Q' generate first
                    nc.scalar.dma_start(out=wvo_sb[:], in_=wvo_d[:])

                cuts, xparts = xhalf

                def xslice(ei, c0, c1):
                    for p in range(len(cuts) - 1):
                        if c0 >= cuts[p] and c1 <= cuts[p + 1]:
                            return xparts[ei][p][:, c0 - cuts[p]:c1 - cuts[p]]
                    raise AssertionError(f"xt slice {c0}:{c1} crosses a piece")

                # ---- Q'T = (X @ Mq)^T for the 16 main blocks ----
                qT = qpool.tile([128, 2, SHARD_Q], f16)
                for cc in range(4):
                    for eo in range(2):
                        pq = pqpool.tile([128, 512], f32)
                        for ei in range(2):
                            nc.tensor.matmul(
                                pq[:],
                                lhsT=mq_sb[:, ei * 2 + eo, :],
                                rhs=xslice(ei, 128 + cc * 512, 128 + (cc + 1) * 512),
                                start=(ei == 0), stop=(ei == 1),
                            )
                        dstq = qT[:, eo, cc * 512:(cc + 1) * 512]
                        if qcopy_mod and (eo * 4 + cc) % qcopy_mod == 0:
                            nc.vector.tensor_copy(dstq, pq[:])
                        else:
                            nc.scalar.activation(dstq, pq[:], Copy)

                # ---- V'aug = [X @ Wvo | 1] and logitsT + exp, interleaved
                # per 2-key-block pair (each pair: one PSUM-bank matmul group
                # + one wide copy / exp). Edge logit blocks use a clamped
                # full-384 query window so exp input is always fully written.
                vaug = vpool.tile([128, CTX, 257], f16)
                nc.gpsimd.memset(vaug[:, :, 256:257], 1.0)
                expT = epool.tile([128, CTX, 384], f16)
                for kb in range(0, CTX, 2):
                    # V' + logits matmuls for the pair, ordered so both
                    # matmuls sharing one stationary xts[ei] slice are
                    # adjacent (halves LDWEIGHTS churn on hardware).
                    pv = pvpool.tile([128, 2, 256], f32)
                    pl = plpool.tile([128, 2, 512], f32)
                    width = []
                    for sl in range(2):
                        j0 = max(1, kb + sl - 1)
                        w = (min(HALF, kb + sl + 1) - j0 + 1) * 128
                        ql = (j0 - 1) * 128
                        width.append(w)
                        for ei in range(2):
                            lhsT = xslice(ei, (kb + sl) * 128, (kb + sl + 1) * 128)
                            nc.tensor.matmul(
                                pv[:, sl, :], lhsT=lhsT, rhs=wvo_sb[:, ei, :],
                                start=(ei == 0), stop=(ei == 1),
                            )
                            nc.tensor.matmul(
                                pl[:, sl, 0:w], lhsT=lhsT, rhs=qT[:, ei, ql:ql + w],
                                start=(ei == 0), stop=(ei == 1),
                            )
                    nc.vector.tensor_copy(vaug[:, kb:kb + 2, 0:256], pv[:])
                    if width[0] == 384 and width[1] == 384:
                        nc.scalar.activation(expT[:, kb:kb + 2, :], pl[:, :, 0:384], Exp)
                    else:
                        for sl in range(2):
                            nc.scalar.activation(
                                expT[:, kb + sl, 0:width[sl]], pl[:, sl, 0:width[sl]], Exp)

                # ---- z = softmax @ V' (+ fused 1/Z) per query block ----
                for j0 in range(1, HALF + 1, zgrp):
                    glen = zgrp
                    zo = zpool.tile([128, zgrp, 256], f16, tag="zo")
                    for j in range(j0, j0 + glen):
                        pz = pzpool.tile([128, 257], f32)
                        for idx, kk in enumerate((j - 1, j, j + 1)):
                            c0 = (j - max(1, kk - 1)) * 128
                            nc.tensor.matmul(
                                pz[:],
                                lhsT=expT[:, kk, c0:c0 + 128],
                                rhs=vaug[:, kk, :],
                                start=(idx == 0), stop=(idx == 2),
                            )
                        rz = rpool.tile([128, 1], f32)
                        nc.vector.reciprocal(rz[:], pz[:, 256:257])
                        dst = zo[:, j - j0, :]
                        if z_act_mod and j % z_act_mod == 0:
                            nc.scalar.activation(dst, pz[:, 0:256], Copy, scale=rz[:])
                        else:
                            nc.vector.tensor_scalar(dst, pz[:, 0:256], rz[:], None, mult)
                    nc.sync.dma_start(
                        out=z_d[s, j0 - 1:j0 - 1 + glen].rearrange("b r c -> r b c"),
                        in_=zo[:, 0:glen, :],
                    )

    if split:
        _split_excess_waits(nc)
    return nc


def _split_excess_waits(nc, max_waits=1):
    """walrus CTRL-class codegen only allows 1 sync wait per instruction;
    move extras onto preceding same-engine NoOps."""
    import concourse.mybir as mybir
    n = 0
    for fn in nc.m.functions:
        for blk in fn.blocks:
            insts = blk.instructions
            i = 0
            while i < len(insts):
                ins = insts[i]
                si = ins.sync_info
                if si is not None and len(si.on_wait) > max_waits:
                    w = list(si.on_wait)
                    nop = mybir.InstNoOp(name=f"I-waitsplit-{n}", ins=[], outs=[])
                    n += 1
                    nop.engine = ins.engine
                    nop.sync_info = mybir.SyncInfo(on_wait=w[:max_waits], on_update=[])
                    ins.sync_info = mybir.SyncInfo(on_wait=w[max_waits:], on_update=si.on_update)
                    insts.insert(i, nop)
                i += 1
    return n


def _prep_inputs(x, Wq, Wk, Wv, Wo):
    """Host-side fold + shard + transpose. Returns per-core input maps."""
    Mq = (Wq.astype(np.float64) @ Wk.astype(np.float64).T / np.sqrt(np.float64(E)))
    Wvo = Wv.astype(np.float64) @ Wo.astype(np.float64)
    Mq16 = Mq.astype(np.float32).astype(F16)
    Wvo16 = Wvo.astype(np.float32).astype(F16)

    mq_chunks = np.empty((128, 4, 128), dtype=F16)
    for ei in range(2):
        for eo in range(2):
            mq_chunks[:, ei * 2 + eo, :] = Mq16[ei * 128:(ei + 1) * 128,
                                                eo * 128:(eo + 1) * 128]
    wvo_chunks = np.empty((128, 2, 256), dtype=F16)
    for ei in range(2):
        wvo_chunks[:, ei, :] = Wvo16[ei * 128:(ei + 1) * 128, :]

    x16 = x.astype(F16)                       # [B, S, E]
    # x_ext per shard: [SHARD_X, E] with halo blocks (zeros at batch edges)
    xt = np.zeros((NCORES, SHPC, 2, 128, SHARD_X), dtype=F16)
    for g in range(NCORES * SHPC):
        b, h = divmod(g, 2)
        c, s = divmod(g, SHPC)
        ext = np.zeros((SHARD_X, E), dtype=F16)
        lo = h * SHARD_Q
        ext[BLOCK:BLOCK + SHARD_Q] = x16[b, lo:lo + SHARD_Q]
        if h == 1:
            ext[:BLOCK] = x16[b, lo - BLOCK:lo]
        else:
            ext[BLOCK + SHARD_Q:] = x16[b, lo + SHARD_Q:lo + SHARD_Q + BLOCK]
        xT = np.ascontiguousarray(ext.T)      # [E, SHARD_X]
        xt[c, s, 0] = xT[:128]
        xt[c, s, 1] = xT[128:]
    return [
        {"xt": xt[c], "mq": mq_chunks, "wvo": wvo_chunks}
        for c in range(NCORES)
    ]


def _run(in_maps, trace=False, trace_kwargs=None):
    from concourse.bass_utils import run_bass_kernel_spmd
    if "nc" not in _CACHE:
        _CACHE["nc"] = _build_module()
    kw = {}
    if trace:
        kw.update(trace=True, trace_cores=list(range(NCORES)))
        if trace_kwargs:
            kw.update(trace_kwargs)
    return run_bass_kernel_spmd(_CACHE["nc"], in_maps, list(range(NCORES)), **kw)


def kernel(x, Wq, Wk, Wv, Wo, bo, _trace=False):
    x = np.asarray(x, dtype=np.float32)
    in_maps = _prep_inputs(
        x,
        np.asarray(Wq, dtype=np.float32), np.asarray(Wk, dtype=np.float32),
        np.asarray(Wv, dtype=np.float32), np.asarray(Wo, dtype=np.float32),
    )
    res = _run(in_maps, trace=_trace)
    kernel.last_result = res

    y = np.empty((B, NB, BLOCK, E), dtype=np.float32)
    for c in range(NCORES):
        zc = res.results[c]["z"]             # [SHPC, HALF, 128, 256]
        for s in range(SHPC):
            g = c * SHPC + s
            b, h = divmod(g, 2)
            y[b, h * HALF:(h + 1) * HALF] = zc[s]
    # reference applies transpose(1,0,2,3).reshape before Wo; Wo acts rowwise
    # so the row permutation commutes -- apply it here on the final rows.
    out = y.transpose(1, 0, 2, 3).reshape(B, S, E) + np.asarray(bo, dtype=np.float32)
    return out
